# revision 22
# baseline (speedup 1.0000x reference)
"""Trainium2 Bass kernel for the NoisyTopK MoE layer (B=2,T=2048,D=1024,H=4096,O=1024,E=8,K=2).

Strategy (expert-parallel with H-split load balancing, 8 cores):
  * Host: compute the full noisy-top2 routing (indices AND softmax gates,
    tiny numpy), gather each expert's tokens.
  * Each expert's FFN is split into two H-halves (hidden dim 4096 -> 2x2048),
    each half on a different core. The 4 heaviest experts pair with the 4
    lightest: core 2i runs (big_i, half0) then (small_i, half0); core 2i+1
    the half1s. Every core thus runs phase A (capacity CA = max big load)
    and phase B (CB = max small load) -- near-perfectly balanced vs. padding
    every core to the global max load.
  * Device (per core, SPMD): for each phase:
      half-FFN: out_half = relu(x @ W1h + b1h) @ W2h, scaled by the token
      gate; fused MM1->MM2 per H-slice, f16 matmuls. Phase A's first chunk
      is XL (6 token-blocks) so the 16MB weight stream (both phases, paced
      in Sync-queue issue order) gets a long runway; PSUM only holds 3
      blocks of MM2 accumulators, so the XL chunk's blocks 3-5 defer their
      MM2 to a dense burst after the m-loop. PE warm-up matmuls run during
      the startup DMA stall so the HAM clock gate is already at 8/8 when
      real work starts.
  * Host: out[tok] += half0 + half1 (gates folded on device; b2 folded on
    host as gates @ b2).
"""

import os
import time

import numpy as np

P = 128
B, T, D, H, O, E = 2, 2048, 1024, 4096, 1024, 8
KD = D // P   # 8  k-tiles over D
NM = H // (2 * P)  # 16 m-slices per H-half
OS = 2        # O-slices of 512
TB = 3        # token-blocks per regular chunk (384 tokens)
XLTB = 6      # token-blocks in phase A's first (weight-streaming) chunk
NWARM = 13    # PE warm-up matmuls (cover the HAM window + startup DMA wait)

_NC_CACHE = {}
LAST_RUN = {}


def _chunks_for(C, xl_first, split_last=False):
    NTB = (C + P - 1) // P
    blocks = [P] * (C // P) + ([C % P] if C % P else [])
    chunks = []
    b0 = 0
    while b0 < NTB:
        n = min(XLTB if (xl_first and b0 == 0) else TB, NTB - b0)
        if split_last and b0 + n == NTB and n > 1:
            n -= 1  # keep the very last chunk a single block (short drain)
        chunks.append((b0, sum(blocks[:b0]), blocks[b0 : b0 + n]))
        b0 += n
    return NTB, blocks, chunks


def _build_nc(CA, CB):
    import concourse.mybir as mybir
    import concourse.tile as tile
    from concourse import bacc

    f32 = mybir.dt.float32
    f16 = mybir.dt.float16
    AF = mybir.ActivationFunctionType

    # Bacc (not plain Bass): its compile() pass splits multi-wait matmuls
    # (HW allows a single sync-wait on the fused LDWEIGHTS+MATMULT).
    nc = bacc.Bacc()

    phases = []
    for ph, C, xl in (("A", CA, True), ("B", CB, False)):
        NTB, blocks, chunks = _chunks_for(C, xl, split_last=(ph == "B"))
        phases.append(
            dict(
                ph=ph,
                C=C,
                NTB=NTB,
                blocks=blocks,
                chunks=chunks,
                xh_d=nc.declare_dram_parameter(
                    f"xh{ph}", [P, KD, C], f16, isOutput=False
                ),
                w1_d=nc.declare_dram_parameter(
                    f"w1s{ph}", [P, NM, KD, P], f16, isOutput=False
                ),
                w2_d=nc.declare_dram_parameter(
                    f"w2s{ph}", [P, NM, O], f16, isOutput=False
                ),
                b1_d=nc.declare_dram_parameter(
                    f"b1s{ph}", [P, NM], f32, isOutput=False
                ),
                g_d=nc.declare_dram_parameter(
                    f"g{ph}", [P, NTB], f32, isOutput=False
                ),
                # f16 output: halves store bytes; quantization (~6e-4 of out
                # scale) is far under the accuracy budget
                out_d=nc.declare_dram_parameter(
                    f"out{ph}", [C, O], f16, isOutput=True
                ),
            )
        )

    with tile.TileContext(nc) as tc:
        with (
            tc.tile_pool(name="singles", bufs=1) as singles,
            tc.tile_pool(name="xpool", bufs=2) as xpool,
            tc.tile_pool(name="hpool", bufs=NM + 1) as hpool,
            tc.tile_pool(name="spool", bufs=4) as spool,
            tc.tile_pool(name="psA", bufs=6, space="PSUM") as psA,
            tc.tile_pool(name="psB", bufs=2, space="PSUM") as psB,
        ):
            # ---- per-phase resident tensors (4+4 MB weights per phase) ----
            for p in phases:
                ph = p["ph"]
                p["w1_sb"] = singles.tile([P, NM, KD, P], f16, name=f"w1sb{ph}")
                p["w2_sb"] = singles.tile([P, NM, O], f16, name=f"w2sb{ph}")
                p["b1_sb"] = singles.tile([P, NM], f32, name=f"b1sb{ph}")
                p["g_sb"] = singles.tile([P, p["NTB"]], f32, name=f"gsb{ph}")

            # ---- PE warm-up: garbage matmuls with no DMA deps so the HAM
            # clock gate reaches 8/8 during the startup DMA stall ----
            wtile = singles.tile([P, TB * P], f16)
            nc.vector.memset(wtile[:], 0.0)
            wps = psB.tile([P, TB * P], f32, tag="mm1ps")
            for _ in range(NWARM):
                nc.tensor.matmul(
                    wps[:], wtile[:, :P], wtile[:], start=True, stop=True
                )

            # DMA instruction issue costs ~610ns on the issuing engine and
            # only ~4 ride in flight, so instruction COUNT (not bandwidth)
            # gates the startup: use few, large transfers.
            def emit_w1_load(p, m, n=1):
                # n consecutive m-slices in one DMA (2KB runs per partition)
                nc.sync.dma_start(
                    p["w1_sb"][:, m : m + n], p["w1_d"][:, m : m + n]
                )

            def emit_w2_load(p, m, n=1):
                nc.sync.dma_start(
                    p["w2_sb"][:, m : m + n, :], p["w2_d"][:, m : m + n, :]
                )

            def emit_x_load(p, ci, nsplit=1):
                # x for one chunk as one (or two) multi-ko DMAs; the tile is
                # [P, KD, ntile] so MM1 slices xs[:, ko, :]
                _, t0c, bsz = p["chunks"][ci]
                nt = sum(bsz)
                ntile = XLTB * P if ci == 0 and p["ph"] == "A" else TB * P
                xs = xpool.tile([P, KD, ntile], f16, tag="xs", name="xs")
                if nsplit == 2:
                    nc.sync.dma_start(
                        xs[:, :2, :nt], p["xh_d"][:, :2, t0c : t0c + nt]
                    )
                    nc.sync.dma_start(
                        xs[:, 2:, :nt], p["xh_d"][:, 2:, t0c : t0c + nt]
                    )
                else:
                    nc.sync.dma_start(
                        xs[:, :, :nt], p["xh_d"][:, :, t0c : t0c + nt]
                    )
                return xs

            pA, pB = phases

            # ---- startup-critical Sync-queue order: x ko0-1, w1 m0, rest
            # of x, w1 m1-5 -- the first matmul needs only the first two
            # transfers (~640KB) ----
            _, t0c0, bsz0 = pA["chunks"][0]
            nt0 = sum(bsz0)
            xs_next = xpool.tile([P, KD, XLTB * P], f16, tag="xs", name="xs")
            # sub0's x first (the m-loop starts on tokens 0-383); the first
            # piece rides the (otherwise idle) Scalar ring so it transfers
            # concurrently with w1 m0 on Sync
            s0 = min(TB * P, nt0)
            nc.scalar.dma_start(xs_next[:, :2, :s0], pA["xh_d"][:, :2, :s0])
            emit_w1_load(pA, 0)
            nc.sync.dma_start(xs_next[:, 2:, :s0], pA["xh_d"][:, 2:, :s0])
            emit_w1_load(pA, 1)
            if nt0 > s0:
                nc.sync.dma_start(
                    xs_next[:, :, s0:nt0], pA["xh_d"][:, :, s0:nt0]
                )
            emit_w1_load(pA, 2, n=2)
            emit_w1_load(pA, 4, n=2)
            nc.scalar.dma_start(pA["b1_sb"][:], pA["b1_d"][:])

            # MM2 trails MM1 by DELTA H-slices: the PE always has independent
            # MM1 work while MM2 waits on relu eviction / psum-slot release.
            # Only the very last chunk of phase B runs the short lag (its
            # drain is the only one not overlapped by following work).
            DELTA_MID, DELTA_LAST = 6, 2

            for pi, p in enumerate(phases):
                chunks = p["chunks"]
                w1_sb, w2_sb, b1_sb, g_sb = (
                    p["w1_sb"], p["w2_sb"], p["b1_sb"], p["g_sb"]
                )
                last_phase = pi == len(phases) - 1
                for ci, (b0c, t0c, bsz) in enumerate(chunks):
                    DELTA = (
                        DELTA_LAST
                        if last_phase and ci == len(chunks) - 1
                        else DELTA_MID
                    )
                    nt = sum(bsz)
                    ntb = len(bsz)
                    bofs = [sum(bsz[:j]) for j in range(ntb)]
                    nlive = min(ntb, TB)  # blocks whose MM2 runs in-loop
                    defer = ntb > nlive  # XL chunk: blocks 3+ deferred
                    xs = xs_next
                    accs = [
                        [
                            psA.tile(
                                [P, 512], f32, tag="acc", name=f"acc_{j}_{osl}"
                            )
                            for osl in range(OS)
                        ]
                        for j in range(nlive)
                    ]
                    # a <128-wide final block would give MM2 a narrow
                    # stationary (disables FWL, +50ns/MM); zero-pad hm so its
                    # MM2s run as full 128-col stationary instead
                    padw = (bofs[-1] + P) - nt if bsz[-1] < P else 0
                    # sub-tiles for MM1 psum ([128, <=384] per bank)
                    subs = []
                    o = 0
                    while o < nt:
                        w = min(TB * P, nt - o)
                        subs.append((o, w))
                        o += w
                    hms = {}
                    for m in range(NM):
                        if pi == 0 and ci == 0 and m == 4:
                            # evict-phase constants on the idle Scalar ring
                            nc.scalar.dma_start(g_sb[:], p["g_d"][:])
                            nc.scalar.dma_start(pB["b1_sb"][:], pB["b1_d"][:])
                            nc.scalar.dma_start(pB["g_sb"][:], pB["g_d"][:])
                        if (m == 4 if ci == 0 and pi == 0 else m == 8):
                            if ci + 1 < len(chunks):
                                # prefetch next chunk's x while this chunk's
                                # m-loop keeps the PE saturated
                                xs_next = emit_x_load(p, ci + 1)
                            elif not last_phase:
                                # phase B's first chunk
                                xs_next = emit_x_load(pB, 0)
                        if pi == 0 and ci == 0:
                            # weight stream, paced by Sync issue order: this
                            # m's phase-A loads first, then one 2-slice DMA
                            # of phase B per m (all of B lands by chunk end;
                            # it is only needed by phase A's end)
                            if 6 <= m + 3 < NM:
                                emit_w1_load(pA, m + 3)
                            emit_w2_load(pA, m)
                            if m % 2 == 0:
                                emit_w1_load(pB, m, n=2)
                            else:
                                emit_w2_load(pB, m - 1, n=2)
                        hm = hpool.tile([P, nt + padw], f16, tag="hm")
                        for so, sw in subs:
                            hps = psB.tile([P, TB * P], f32, tag="mm1ps")
                            hw = hps[:, :sw]
                            for ko in range(KD):
                                nc.tensor.matmul(
                                    hw,
                                    w1_sb[:, m, ko, :],
                                    xs[:, ko, so : so + sw],
                                    start=(ko == 0),
                                    stop=(ko == KD - 1),
                                )
                            nc.scalar.activation(
                                hm[:, so : so + sw],
                                hw,
                                AF.Relu,
                                bias=b1_sb[:, m : m + 1],
                            )
                        if padw:
                            nc.vector.memset(hm[:, nt : nt + padw], 0.0)
                        hms[m] = hm
                        if m >= DELTA:
                            mm = m - DELTA
                            hm2 = hms[mm] if defer else hms.pop(mm)
                            for j in range(nlive):
                                bs = P if j == ntb - 1 and padw else bsz[j]
                                for osl in range(OS):
                                    nc.tensor.matmul(
                                        accs[j][osl][:bs],
                                        hm2[:, bofs[j] : bofs[j] + bs],
                                        w2_sb[:, mm, osl * 512 : (osl + 1) * 512],
                                        start=(mm == 0),
                                        stop=(mm == NM - 1),
                                    )

                    def evict(j, bs, acc2):
                        # evict: acc * gate -> DRAM (f16, 2-queue split); the
                        # b2 bias term is added on the host. The two O-halves
                        # run on different engines (DVE / ACT) and the two
                        # row-half stores issue from different engines, so
                        # the kernel tail is not a serial chain.
                        st = spool.tile([P, O], f16, tag="st")
                        gcol = g_sb[:bs, b0c + j : b0c + j + 1]
                        nc.vector.tensor_scalar_mul(
                            st[:bs, :512], acc2[0][:bs], gcol
                        )
                        nc.scalar.activation(
                            st[:bs, 512:], acc2[1][:bs], AF.Copy, scale=gcol
                        )
                        g0 = t0c + bofs[j]
                        h1 = bs // 2
                        nc.sync.dma_start(
                            p["out_d"][g0 : g0 + h1, :], st[:h1, :]
                        )
                        nc.scalar.dma_start(
                            p["out_d"][g0 + h1 : g0 + bs, :], st[h1:bs, :]
                        )

                    # ---- pipeline drain, block-major: finish block j's
                    # accumulation, then evict it while block j+1 drains ----
                    for j in range(nlive):
                        bs = bsz[j]
                        bsm = P if j == ntb - 1 and padw else bs
                        for mm in range(NM - DELTA, NM):
                            hm2 = hms[mm]
                            for osl in range(OS):
                                nc.tensor.matmul(
                                    accs[j][osl][:bsm],
                                    hm2[:, bofs[j] : bofs[j] + bsm],
                                    w2_sb[:, mm, osl * 512 : (osl + 1) * 512],
                                    start=(mm == 0),
                                    stop=(mm == NM - 1),
                                )
                        evict(j, bs, accs[j])
                    # ---- deferred blocks (XL chunk): dense MM2 burst ----
                    for j in range(nlive, ntb):
                        bs = bsz[j]
                        bsm = P if j == ntb - 1 and padw else bs
                        acc2 = [
                            psA.tile([P, 512], f32, tag="acc", name=f"accd_{osl}")
                            for osl in range(OS)
                        ]
                        for mm in range(NM):
                            hm2 = hms[mm]
                            for osl in range(OS):
                                nc.tensor.matmul(
                                    acc2[osl][:bsm],
                                    hm2[:, bofs[j] : bofs[j] + bsm],
                                    w2_sb[:, mm, osl * 512 : (osl + 1) * 512],
                                    start=(mm == 0),
                                    stop=(mm == NM - 1),
                                )
                        evict(j, bs, acc2)
                    hms.clear()

    nc.finalize()
    return nc


def _routing_host(xf, nf, Wg, bg, Wn, bn):
    """Top-2 expert mask AND the sparse softmax gates per token."""
    logits = xf @ Wg + bg
    nl = xf @ Wn + bn
    sp = np.logaddexp(0.0, nl)
    noisy = logits + nf * sp
    order = np.argpartition(-noisy, 2, axis=1)[:, :2]
    mask = np.zeros(noisy.shape, dtype=bool)
    mask[np.arange(noisy.shape[0])[:, None], order] = True
    # softmax over the two selected logits (matches reference: softmax of
    # the -inf-masked logits, then L1-normalize -- a numeric no-op)
    neg = np.where(mask, noisy, -np.inf)
    mx = neg.max(axis=1, keepdims=True)
    ex = np.exp(neg - mx)
    gates = ex / ex.sum(axis=1, keepdims=True)
    gates[~mask] = 0.0
    return mask, gates.astype(np.float32)


def _prep_phase(xf, gates, idx, C, W1h, b1h, W2h, e):
    """Per-core inputs for one phase: expert e's tokens, one H-half of its
    FFN (W1h [D, 2048], b1h [2048], W2h [2048, O])."""
    n = len(idx)
    x_g = np.zeros((C, D), np.float32)
    x_g[:n] = xf[idx]
    NTB = (C + P - 1) // P
    g_g = np.zeros((NTB * P,), np.float32)
    g_g[:n] = gates[idx, e]
    xh = np.ascontiguousarray(
        x_g.reshape(C, KD, P).transpose(2, 1, 0)
    ).astype(np.float16)
    return {
        "xh": xh,
        "w1s": np.ascontiguousarray(
            W1h.reshape(KD, P, NM, P).transpose(1, 2, 0, 3)
        ).astype(np.float16),
        "w2s": np.ascontiguousarray(
            W2h.reshape(NM, P, O).transpose(1, 0, 2)
        ).astype(np.float16),
        "b1s": np.ascontiguousarray(b1h.reshape(NM, P).T),
        "g": np.ascontiguousarray(g_g.reshape(NTB, P).T),
    }


def kernel(x, noise, Wg, bg, Wn, bn, W1, b1, W2, b2):
    from concourse.bass_utils import run_bass_kernel_spmd

    x = np.asarray(x, np.float32)
    noise = np.asarray(noise, np.float32)
    Wg = np.asarray(Wg, np.float32)
    bg = np.asarray(bg, np.float32)
    Wn = np.asarray(Wn, np.float32)
    bn = np.asarray(bn, np.float32)
    W1 = np.asarray(W1, np.float32)
    b1 = np.asarray(b1, np.float32)
    W2 = np.asarray(W2, np.float32)
    b2 = np.asarray(b2, np.float32)

    Bx, Tx, _ = x.shape
    ntok = Bx * Tx
    xf = x.reshape(ntok, D)
    nf = noise.reshape(ntok, E)

    mask, gates = _routing_host(xf, nf, Wg, bg, Wn, bn)
    idx = [np.nonzero(mask[:, e])[0] for e in range(E)]
    loads = np.array([len(i) for i in idx])

    # 4 heaviest experts -> phase A, 4 lightest -> phase B; each expert's
    # H-halves land on cores (2i, 2i+1)
    order = np.argsort(-loads, kind="stable")
    bigs, smalls = order[:4], order[4:]
    CA = max(2 * P, int(loads[bigs].max()))
    CB = max(2 * P, int(loads[smalls].max()))

    if (CA, CB) not in _NC_CACHE:
        _NC_CACHE[(CA, CB)] = _build_nc(CA, CB)
    nc = _NC_CACHE[(CA, CB)]

    Hh = H // 2
    in_maps = []
    for i in range(4):
        a, b = int(bigs[i]), int(smalls[i])
        for half in range(2):
            h0 = half * Hh
            mA = _prep_phase(
                xf, gates, idx[a], CA,
                W1[a][:, h0 : h0 + Hh], b1[a][h0 : h0 + Hh],
                W2[a][h0 : h0 + Hh, :], a,
            )
            mB = _prep_phase(
                xf, gates, idx[b], CB,
                W1[b][:, h0 : h0 + Hh], b1[b][h0 : h0 + Hh],
                W2[b][h0 : h0 + Hh, :], b,
            )
            in_maps.append(
                {k + "A": v for k, v in mA.items()}
                | {k + "B": v for k, v in mB.items()}
            )

    trace = bool(os.environ.get("MOE_TRACE"))
    if trace:
        try:
            import antenv.axon_hooks  # noqa: F401 (bass_utils needs it when tracing)
        except ImportError:
            # shim the missing module and register the ctypes NTFF hook
            try:
                import sys
                import types

                import antenv
                from trn_agent_boot.trn_boot import _ntff_profile_via_ctypes

                mod = types.ModuleType("antenv.axon_hooks")
                _h = [None]
                mod.set_axon_ntff_profile_hook = lambda h: _h.__setitem__(0, h)
                mod.get_axon_ntff_profile_hook = lambda: _h[0]
                antenv.axon_hooks = mod
                sys.modules["antenv.axon_hooks"] = mod
                mod.set_axon_ntff_profile_hook(
                    _ntff_profile_via_ctypes("/opt/axon/libaxon_pjrt.so")
                )
            except Exception:
                trace = False
    t0 = time.time()
    res = run_bass_kernel_spmd(nc, in_maps, list(range(E)), trace=trace)
    t1 = time.time()
    LAST_RUN.clear()
    LAST_RUN.update(
        wall_s=t1 - t0,
        exec_time_ns=res.exec_time_ns,
        trace=res.instructions_and_trace[1] if res.instructions_and_trace else None,
    )

    # the gated b2 bias term, dropped from the device evict: sum_e g_e*b2_e
    out = gates @ b2.astype(np.float32)
    for i in range(4):
        for ph, e, C in (("A", int(bigs[i]), CA), ("B", int(smalls[i]), CB)):
            n = len(idx[e])
            acc = np.zeros((n, O), np.float32)
            for half in range(2):
                y = res.results[2 * i + half][f"out{ph}"].reshape(C, O)
                acc += y[:n].astype(np.float32)
            out[idx[e]] += acc
    return out.reshape(Bx, Tx, O)


# revision 23
# speedup vs baseline: 1.0077x; 1.0077x over previous
"""Trainium2 Bass kernel for the NoisyTopK MoE layer (B=2,T=2048,D=1024,H=4096,O=1024,E=8,K=2).

Strategy (expert-parallel with H-split load balancing, 8 cores):
  * Host: compute the full noisy-top2 routing (indices AND softmax gates,
    tiny numpy), gather each expert's tokens.
  * Each expert's FFN is split into two H-halves (hidden dim 4096 -> 2x2048),
    each half on a different core. The 4 heaviest experts pair with the 4
    lightest: core 2i runs (big_i, half0) then (small_i, half0); core 2i+1
    the half1s. Every core thus runs phase A (capacity CA = max big load)
    and phase B (CB = max small load) -- near-perfectly balanced vs. padding
    every core to the global max load.
  * Device (per core, SPMD): for each phase:
      half-FFN: out_half = relu(x @ W1h + b1h) @ W2h, scaled by the token
      gate; fused MM1->MM2 per H-slice, f16 matmuls. Phase A's first chunk
      is XL (6 token-blocks) so the 16MB weight stream (both phases, paced
      in Sync-queue issue order) gets a long runway; PSUM only holds 3
      blocks of MM2 accumulators, so the XL chunk's blocks 3-5 defer their
      MM2 to a dense burst after the m-loop. PE warm-up matmuls run during
      the startup DMA stall so the HAM clock gate is already at 8/8 when
      real work starts.
  * Host: out[tok] += half0 + half1 (gates folded on device; b2 folded on
    host as gates @ b2).
"""

import os
import time

import numpy as np

P = 128
B, T, D, H, O, E = 2, 2048, 1024, 4096, 1024, 8
KD = D // P   # 8  k-tiles over D
NM = H // (2 * P)  # 16 m-slices per H-half
OS = 2        # O-slices of 512
TB = 3        # token-blocks per regular chunk (384 tokens)
XLTB = 6      # token-blocks in phase A's first (weight-streaming) chunk
NWARM = 20    # PE warm-up matmuls (cover the HAM window + startup DMA wait)

_NC_CACHE = {}
LAST_RUN = {}


def _chunks_for(C, xl_first, split_last=False):
    NTB = (C + P - 1) // P
    blocks = [P] * (C // P) + ([C % P] if C % P else [])
    chunks = []
    b0 = 0
    while b0 < NTB:
        n = min(XLTB if (xl_first and b0 == 0) else TB, NTB - b0)
        if split_last and b0 + n == NTB and n > 1:
            n -= 1  # keep the very last chunk a single block (short drain)
        chunks.append((b0, sum(blocks[:b0]), blocks[b0 : b0 + n]))
        b0 += n
    return NTB, blocks, chunks


def _build_nc(CA, CB):
    import concourse.mybir as mybir
    import concourse.tile as tile
    from concourse import bacc

    f32 = mybir.dt.float32
    f16 = mybir.dt.float16
    AF = mybir.ActivationFunctionType

    # Bacc (not plain Bass): its compile() pass splits multi-wait matmuls
    # (HW allows a single sync-wait on the fused LDWEIGHTS+MATMULT).
    nc = bacc.Bacc()

    phases = []
    for ph, C, xl in (("A", CA, True), ("B", CB, False)):
        NTB, blocks, chunks = _chunks_for(C, xl, split_last=(ph == "B"))
        phases.append(
            dict(
                ph=ph,
                C=C,
                NTB=NTB,
                blocks=blocks,
                chunks=chunks,
                xh_d=nc.declare_dram_parameter(
                    f"xh{ph}", [P, KD, C], f16, isOutput=False
                ),
                w1_d=nc.declare_dram_parameter(
                    f"w1s{ph}", [P, NM, KD, P], f16, isOutput=False
                ),
                w2_d=nc.declare_dram_parameter(
                    f"w2s{ph}", [P, NM, O], f16, isOutput=False
                ),
                b1_d=nc.declare_dram_parameter(
                    f"b1s{ph}", [P, NM], f32, isOutput=False
                ),
                g_d=nc.declare_dram_parameter(
                    f"g{ph}", [P, NTB], f32, isOutput=False
                ),
                # f16 output: halves store bytes; quantization (~6e-4 of out
                # scale) is far under the accuracy budget
                out_d=nc.declare_dram_parameter(
                    f"out{ph}", [C, O], f16, isOutput=True
                ),
            )
        )

    with tile.TileContext(nc) as tc:
        with (
            tc.tile_pool(name="singles", bufs=1) as singles,
            tc.tile_pool(name="xpool", bufs=2) as xpool,
            tc.tile_pool(name="hpool", bufs=NM + 1) as hpool,
            tc.tile_pool(name="spool", bufs=4) as spool,
            tc.tile_pool(name="psA", bufs=6, space="PSUM") as psA,
            tc.tile_pool(name="psB", bufs=2, space="PSUM") as psB,
        ):
            # ---- per-phase resident tensors (4+4 MB weights per phase) ----
            for p in phases:
                ph = p["ph"]
                p["w1_sb"] = singles.tile([P, NM, KD, P], f16, name=f"w1sb{ph}")
                p["w2_sb"] = singles.tile([P, NM, O], f16, name=f"w2sb{ph}")
                p["b1_sb"] = singles.tile([P, NM], f32, name=f"b1sb{ph}")
                p["g_sb"] = singles.tile([P, p["NTB"]], f32, name=f"gsb{ph}")

            # ---- PE warm-up: garbage matmuls with no DMA deps so the HAM
            # clock gate reaches 8/8 during the startup DMA stall ----
            wtile = singles.tile([P, TB * P], f16)
            nc.vector.memset(wtile[:], 0.0)
            wps = psB.tile([P, TB * P], f32, tag="mm1ps")
            for _ in range(NWARM):
                nc.tensor.matmul(
                    wps[:], wtile[:, :P], wtile[:], start=True, stop=True
                )

            # DMA instruction issue costs ~610ns on the issuing engine and
            # only ~4 ride in flight, so instruction COUNT (not bandwidth)
            # gates the startup: use few, large transfers.
            def emit_w1_load(p, m, n=1):
                # n consecutive m-slices in one DMA (2KB runs per partition)
                nc.sync.dma_start(
                    p["w1_sb"][:, m : m + n], p["w1_d"][:, m : m + n]
                )

            def emit_w2_load(p, m, n=1):
                nc.sync.dma_start(
                    p["w2_sb"][:, m : m + n, :], p["w2_d"][:, m : m + n, :]
                )

            def emit_x_load(p, ci, nsplit=1):
                # x for one chunk as one (or two) multi-ko DMAs; the tile is
                # [P, KD, ntile] so MM1 slices xs[:, ko, :]
                _, t0c, bsz = p["chunks"][ci]
                nt = sum(bsz)
                ntile = XLTB * P if ci == 0 and p["ph"] == "A" else TB * P
                xs = xpool.tile([P, KD, ntile], f16, tag="xs", name="xs")
                if nsplit == 2:
                    nc.sync.dma_start(
                        xs[:, :2, :nt], p["xh_d"][:, :2, t0c : t0c + nt]
                    )
                    nc.sync.dma_start(
                        xs[:, 2:, :nt], p["xh_d"][:, 2:, t0c : t0c + nt]
                    )
                else:
                    nc.sync.dma_start(
                        xs[:, :, :nt], p["xh_d"][:, :, t0c : t0c + nt]
                    )
                return xs

            pA, pB = phases

            # ---- startup-critical Sync-queue order: x ko0-1, w1 m0, rest
            # of x, w1 m1-5 -- the first matmul needs only the first two
            # transfers (~640KB) ----
            _, t0c0, bsz0 = pA["chunks"][0]
            nt0 = sum(bsz0)
            xs_next = xpool.tile([P, KD, XLTB * P], f16, tag="xs", name="xs")
            # sub0's x first (the m-loop starts on tokens 0-383); the first
            # piece rides the (otherwise idle) Scalar ring so it transfers
            # concurrently with w1 m0 on Sync
            s0 = min(TB * P, nt0)
            nc.scalar.dma_start(xs_next[:, :2, :s0], pA["xh_d"][:, :2, :s0])
            emit_w1_load(pA, 0)
            nc.sync.dma_start(xs_next[:, 2:, :s0], pA["xh_d"][:, 2:, :s0])
            emit_w1_load(pA, 1)
            if nt0 > s0:
                nc.sync.dma_start(
                    xs_next[:, :, s0:nt0], pA["xh_d"][:, :, s0:nt0]
                )
            emit_w1_load(pA, 2, n=2)
            emit_w1_load(pA, 4, n=2)
            nc.scalar.dma_start(pA["b1_sb"][:], pA["b1_d"][:])

            # MM2 trails MM1 by DELTA H-slices: the PE always has independent
            # MM1 work while MM2 waits on relu eviction / psum-slot release.
            # Only the very last chunk of phase B runs the short lag (its
            # drain is the only one not overlapped by following work).
            DELTA_MID, DELTA_LAST = 6, 2

            for pi, p in enumerate(phases):
                chunks = p["chunks"]
                w1_sb, w2_sb, b1_sb, g_sb = (
                    p["w1_sb"], p["w2_sb"], p["b1_sb"], p["g_sb"]
                )
                last_phase = pi == len(phases) - 1
                for ci, (b0c, t0c, bsz) in enumerate(chunks):
                    DELTA = (
                        DELTA_LAST
                        if last_phase and ci == len(chunks) - 1
                        else DELTA_MID
                    )
                    nt = sum(bsz)
                    ntb = len(bsz)
                    bofs = [sum(bsz[:j]) for j in range(ntb)]
                    nlive = min(ntb, TB)  # blocks whose MM2 runs in-loop
                    defer = ntb > nlive  # XL chunk: blocks 3+ deferred
                    xs = xs_next
                    accs = [
                        [
                            psA.tile(
                                [P, 512], f32, tag="acc", name=f"acc_{j}_{osl}"
                            )
                            for osl in range(OS)
                        ]
                        for j in range(nlive)
                    ]
                    # a <128-wide final block would give MM2 a narrow
                    # stationary (disables FWL, +50ns/MM); zero-pad hm so its
                    # MM2s run as full 128-col stationary instead
                    padw = (bofs[-1] + P) - nt if bsz[-1] < P else 0
                    # sub-tiles for MM1 psum ([128, <=384] per bank)
                    subs = []
                    o = 0
                    while o < nt:
                        w = min(TB * P, nt - o)
                        subs.append((o, w))
                        o += w
                    hms = {}
                    for m in range(NM):
                        if pi == 0 and ci == 0 and m == 4:
                            # evict-phase constants on the idle Scalar ring
                            nc.scalar.dma_start(g_sb[:], p["g_d"][:])
                            nc.scalar.dma_start(pB["b1_sb"][:], pB["b1_d"][:])
                            nc.scalar.dma_start(pB["g_sb"][:], pB["g_d"][:])
                        if (m == 4 if ci == 0 and pi == 0 else m == 8):
                            if ci + 1 < len(chunks):
                                # prefetch next chunk's x while this chunk's
                                # m-loop keeps the PE saturated
                                xs_next = emit_x_load(p, ci + 1)
                            elif not last_phase:
                                # phase B's first chunk
                                xs_next = emit_x_load(pB, 0)
                        if pi == 0 and ci == 0:
                            # weight stream, paced by Sync issue order: this
                            # m's phase-A loads first, then one 2-slice DMA
                            # of phase B per m (all of B lands by chunk end;
                            # it is only needed by phase A's end)
                            if 6 <= m + 3 < NM:
                                emit_w1_load(pA, m + 3)
                            emit_w2_load(pA, m)
                            if m % 2 == 0:
                                emit_w1_load(pB, m, n=2)
                            else:
                                emit_w2_load(pB, m - 1, n=2)
                        hm = hpool.tile([P, nt + padw], f16, tag="hm")
                        for so, sw in subs:
                            hps = psB.tile([P, TB * P], f32, tag="mm1ps")
                            hw = hps[:, :sw]
                            for ko in range(KD):
                                nc.tensor.matmul(
                                    hw,
                                    w1_sb[:, m, ko, :],
                                    xs[:, ko, so : so + sw],
                                    start=(ko == 0),
                                    stop=(ko == KD - 1),
                                )
                            nc.scalar.activation(
                                hm[:, so : so + sw],
                                hw,
                                AF.Relu,
                                bias=b1_sb[:, m : m + 1],
                            )
                        if padw:
                            nc.vector.memset(hm[:, nt : nt + padw], 0.0)
                        hms[m] = hm
                        if m >= DELTA:
                            mm = m - DELTA
                            hm2 = hms[mm] if defer else hms.pop(mm)
                            for j in range(nlive):
                                bs = P if j == ntb - 1 and padw else bsz[j]
                                for osl in range(OS):
                                    nc.tensor.matmul(
                                        accs[j][osl][:bs],
                                        hm2[:, bofs[j] : bofs[j] + bs],
                                        w2_sb[:, mm, osl * 512 : (osl + 1) * 512],
                                        start=(mm == 0),
                                        stop=(mm == NM - 1),
                                    )

                    def evict(j, bs, acc2):
                        # evict: acc * gate -> DRAM (f16, 2-queue split); the
                        # b2 bias term is added on the host. The two O-halves
                        # run on different engines (DVE / ACT) and the two
                        # row-half stores issue from different engines, so
                        # the kernel tail is not a serial chain.
                        st = spool.tile([P, O], f16, tag="st")
                        gcol = g_sb[:bs, b0c + j : b0c + j + 1]
                        nc.vector.tensor_scalar_mul(
                            st[:bs, :512], acc2[0][:bs], gcol
                        )
                        nc.scalar.activation(
                            st[:bs, 512:], acc2[1][:bs], AF.Copy, scale=gcol
                        )
                        g0 = t0c + bofs[j]
                        h1 = bs // 2
                        nc.sync.dma_start(
                            p["out_d"][g0 : g0 + h1, :], st[:h1, :]
                        )
                        nc.scalar.dma_start(
                            p["out_d"][g0 + h1 : g0 + bs, :], st[h1:bs, :]
                        )

                    # ---- pipeline drain, block-major: finish block j's
                    # accumulation, then evict it while block j+1 drains ----
                    for j in range(nlive):
                        bs = bsz[j]
                        bsm = P if j == ntb - 1 and padw else bs
                        for mm in range(NM - DELTA, NM):
                            hm2 = hms[mm]
                            for osl in range(OS):
                                nc.tensor.matmul(
                                    accs[j][osl][:bsm],
                                    hm2[:, bofs[j] : bofs[j] + bsm],
                                    w2_sb[:, mm, osl * 512 : (osl + 1) * 512],
                                    start=(mm == 0),
                                    stop=(mm == NM - 1),
                                )
                        evict(j, bs, accs[j])
                    # ---- deferred blocks (XL chunk): dense MM2 burst ----
                    for j in range(nlive, ntb):
                        bs = bsz[j]
                        bsm = P if j == ntb - 1 and padw else bs
                        acc2 = [
                            psA.tile([P, 512], f32, tag="acc", name=f"accd_{osl}")
                            for osl in range(OS)
                        ]
                        for mm in range(NM):
                            hm2 = hms[mm]
                            for osl in range(OS):
                                nc.tensor.matmul(
                                    acc2[osl][:bsm],
                                    hm2[:, bofs[j] : bofs[j] + bsm],
                                    w2_sb[:, mm, osl * 512 : (osl + 1) * 512],
                                    start=(mm == 0),
                                    stop=(mm == NM - 1),
                                )
                        evict(j, bs, acc2)
                    hms.clear()

    nc.finalize()
    return nc


def _routing_host(xf, nf, Wg, bg, Wn, bn):
    """Top-2 expert mask AND the sparse softmax gates per token."""
    logits = xf @ Wg + bg
    nl = xf @ Wn + bn
    sp = np.logaddexp(0.0, nl)
    noisy = logits + nf * sp
    order = np.argpartition(-noisy, 2, axis=1)[:, :2]
    mask = np.zeros(noisy.shape, dtype=bool)
    mask[np.arange(noisy.shape[0])[:, None], order] = True
    # softmax over the two selected logits (matches reference: softmax of
    # the -inf-masked logits, then L1-normalize -- a numeric no-op)
    neg = np.where(mask, noisy, -np.inf)
    mx = neg.max(axis=1, keepdims=True)
    ex = np.exp(neg - mx)
    gates = ex / ex.sum(axis=1, keepdims=True)
    gates[~mask] = 0.0
    return mask, gates.astype(np.float32)


def _prep_phase(xf, gates, idx, C, W1h, b1h, W2h, e):
    """Per-core inputs for one phase: expert e's tokens, one H-half of its
    FFN (W1h [D, 2048], b1h [2048], W2h [2048, O])."""
    n = len(idx)
    x_g = np.zeros((C, D), np.float32)
    x_g[:n] = xf[idx]
    NTB = (C + P - 1) // P
    g_g = np.zeros((NTB * P,), np.float32)
    g_g[:n] = gates[idx, e]
    xh = np.ascontiguousarray(
        x_g.reshape(C, KD, P).transpose(2, 1, 0)
    ).astype(np.float16)
    return {
        "xh": xh,
        "w1s": np.ascontiguousarray(
            W1h.reshape(KD, P, NM, P).transpose(1, 2, 0, 3)
        ).astype(np.float16),
        "w2s": np.ascontiguousarray(
            W2h.reshape(NM, P, O).transpose(1, 0, 2)
        ).astype(np.float16),
        "b1s": np.ascontiguousarray(b1h.reshape(NM, P).T),
        "g": np.ascontiguousarray(g_g.reshape(NTB, P).T),
    }


def kernel(x, noise, Wg, bg, Wn, bn, W1, b1, W2, b2):
    from concourse.bass_utils import run_bass_kernel_spmd

    x = np.asarray(x, np.float32)
    noise = np.asarray(noise, np.float32)
    Wg = np.asarray(Wg, np.float32)
    bg = np.asarray(bg, np.float32)
    Wn = np.asarray(Wn, np.float32)
    bn = np.asarray(bn, np.float32)
    W1 = np.asarray(W1, np.float32)
    b1 = np.asarray(b1, np.float32)
    W2 = np.asarray(W2, np.float32)
    b2 = np.asarray(b2, np.float32)

    Bx, Tx, _ = x.shape
    ntok = Bx * Tx
    xf = x.reshape(ntok, D)
    nf = noise.reshape(ntok, E)

    mask, gates = _routing_host(xf, nf, Wg, bg, Wn, bn)
    idx = [np.nonzero(mask[:, e])[0] for e in range(E)]
    loads = np.array([len(i) for i in idx])

    # 4 heaviest experts -> phase A, 4 lightest -> phase B; each expert's
    # H-halves land on cores (2i, 2i+1)
    order = np.argsort(-loads, kind="stable")
    bigs, smalls = order[:4], order[4:]
    CA = max(2 * P, int(loads[bigs].max()))
    CB = max(2 * P, int(loads[smalls].max()))

    if (CA, CB) not in _NC_CACHE:
        _NC_CACHE[(CA, CB)] = _build_nc(CA, CB)
    nc = _NC_CACHE[(CA, CB)]

    Hh = H // 2
    in_maps = []
    for i in range(4):
        a, b = int(bigs[i]), int(smalls[i])
        for half in range(2):
            h0 = half * Hh
            mA = _prep_phase(
                xf, gates, idx[a], CA,
                W1[a][:, h0 : h0 + Hh], b1[a][h0 : h0 + Hh],
                W2[a][h0 : h0 + Hh, :], a,
            )
            mB = _prep_phase(
                xf, gates, idx[b], CB,
                W1[b][:, h0 : h0 + Hh], b1[b][h0 : h0 + Hh],
                W2[b][h0 : h0 + Hh, :], b,
            )
            in_maps.append(
                {k + "A": v for k, v in mA.items()}
                | {k + "B": v for k, v in mB.items()}
            )

    trace = bool(os.environ.get("MOE_TRACE"))
    if trace:
        try:
            import antenv.axon_hooks  # noqa: F401 (bass_utils needs it when tracing)
        except ImportError:
            # shim the missing module and register the ctypes NTFF hook
            try:
                import sys
                import types

                import antenv
                from trn_agent_boot.trn_boot import _ntff_profile_via_ctypes

                mod = types.ModuleType("antenv.axon_hooks")
                _h = [None]
                mod.set_axon_ntff_profile_hook = lambda h: _h.__setitem__(0, h)
                mod.get_axon_ntff_profile_hook = lambda: _h[0]
                antenv.axon_hooks = mod
                sys.modules["antenv.axon_hooks"] = mod
                mod.set_axon_ntff_profile_hook(
                    _ntff_profile_via_ctypes("/opt/axon/libaxon_pjrt.so")
                )
            except Exception:
                trace = False
    t0 = time.time()
    res = run_bass_kernel_spmd(nc, in_maps, list(range(E)), trace=trace)
    t1 = time.time()
    LAST_RUN.clear()
    LAST_RUN.update(
        wall_s=t1 - t0,
        exec_time_ns=res.exec_time_ns,
        trace=res.instructions_and_trace[1] if res.instructions_and_trace else None,
    )

    # the gated b2 bias term, dropped from the device evict: sum_e g_e*b2_e
    out = gates @ b2.astype(np.float32)
    for i in range(4):
        for ph, e, C in (("A", int(bigs[i]), CA), ("B", int(smalls[i]), CB)):
            n = len(idx[e])
            acc = np.zeros((n, O), np.float32)
            for half in range(2):
                y = res.results[2 * i + half][f"out{ph}"].reshape(C, O)
                acc += y[:n].astype(np.float32)
            out[idx[e]] += acc
    return out.reshape(Bx, Tx, O)
